# revision 1
# baseline (speedup 1.0000x reference)
"""Trainium2 Bass kernel for a dense transformer attention layer.

Computes, for x:[2,2048,1024] (B=2, T=2048, D=1024, H=16 heads, dk=64,
dff=4096):
    q,k,v = split_heads(x@Wq+bq), ...     (per-head dims 64)
    attn  = softmax(causal_mask(q k^T / 8)) v
    x1    = LN(x + 2*attn; g1, be1)
    out   = LN(x1 + 2*(relu(x1@W1+b1)@W2+b2); g2, be2)

Distribution over 8 NeuronCores:
  - QKV + attention: head-parallel (2 heads per core), all tokens.
    Identical causal loop structure on every core (SPMD-safe).
  - Two AllToAlls (one per local head) reshard attention output from
    head-major to token-major; the first overlaps with the second
    head's attention compute. Payload rows carry the unnormalized
    attention plus the softmax denominator row; normalization happens
    after the collective, fused into the residual-add.
  - LN1 + FFN + LN2: token-parallel (512 tokens per core, full dff).

On-chip layout is feature-major ([channels, tokens]) everywhere, so no
activation transposes are needed for matmuls; attention scores are
computed transposed ([k, q]) so the softmax denominator falls out of the
PV matmul via an extra ones-column in the token-major V tiles.
x streams in 512-token chunks so QKV matmuls start ~5us into the
kernel instead of waiting for the full 8MB activation load.
"""
import os
import math
from contextlib import ExitStack

import numpy as np

import concourse.bass as bass
import concourse.tile as tile
from concourse import bacc, mybir
from concourse.bass_utils import run_bass_kernel_spmd

F32 = mybir.dt.float32
F32R = mybir.dt.float32r
BF16 = mybir.dt.bfloat16
F8 = mybir.dt.float8e4
BF = np.dtype("bfloat16")

NCORES = 8
B, T, C, DK, H, DFF = 2, 2048, 1024, 64, 16, 4096
TOK = B * T            # 4096 tokens
TSL = TOK // NCORES    # 512 tokens per core (post-attention shard)
CT = C // 128          # 8 channel tiles
NCH = TOK // 512       # 8 token chunks
EPS = 1e-5

_CACHE = {}
LAST_EXEC_NS = None


def _emit(nc, causal: bool, masked: bool, sim: bool = False, reps: int = 1):
    """Emit the SPMD program. causal: skip/selective-mask causal blocks.
    masked: add a generic additive mask input (maskT, [k,q] layout).
    sim: replace collectives with local DRAM copies (TimelineSim has no
    collective cost model). reps: run the whole layer this many times
    (benchmark amortization; output is rewritten identically)."""
    dt_in = nc.dram_tensor
    xT = dt_in("xT", [CT, 128, TOK], BF16, kind="ExternalInput").ap()
    xres = dt_in("xres", [CT, 128, TSL], F32, kind="ExternalInput").ap()
    wq = dt_in("wq", [CT, 128, 128], BF16, kind="ExternalInput").ap()
    wk = dt_in("wk", [CT, 128, 128], BF16, kind="ExternalInput").ap()
    wv = dt_in("wv", [CT, 128, 128], BF16, kind="ExternalInput").ap()
    bqkv = dt_in("bqkv", [128, 3], F32, kind="ExternalInput").ap()
    w1 = dt_in("w1", [C, DFF], BF16, kind="ExternalInput").ap()
    b1 = dt_in("b1", [128, DFF // 128], F32, kind="ExternalInput").ap()
    w2 = dt_in("w2", [DFF, C], BF16, kind="ExternalInput").ap()
    b2x2 = dt_in("b2x2", [128, CT], F32, kind="ExternalInput").ap()
    g1v = dt_in("g1v", [128, CT], F32, kind="ExternalInput").ap()
    be1v = dt_in("be1v", [128, CT], F32, kind="ExternalInput").ap()
    g2v = dt_in("g2v", [128, CT], F32, kind="ExternalInput").ap()
    be2v = dt_in("be2v", [128, CT], F32, kind="ExternalInput").ap()
    amask = None
    if masked:
        # additive mask, transposed: amask[kt][k 128, q 2048] (bf16, 0/-30000)
        amask = dt_in("amask", [T // 128, 128, T], BF16,
                      kind="ExternalInput").ap()
    out = dt_in("out", [CT, 128, TSL], F32, kind="ExternalOutput").ap()

    NQC = T // 512  # 4 q-chunks of 512 per batch

    with tile.TileContext(nc) as tc, ExitStack() as ctx:
        persist = ctx.enter_context(tc.tile_pool(name="persist", bufs=1))
        dram = ctx.enter_context(tc.tile_pool(name="dram", bufs=1,
                                              space="DRAM"))

        # ---- persistent SBUF tensors (small; live whole kernel) ----
        ident = persist.tile([128, 128], BF16, name="ident", tag="ident")
        ones128 = persist.tile([128, 128], F32R, name="ones128", tag="ones128")
        onesf = persist.tile([128, 128], F32, name="onesf", tag="onesf")
        bias3 = persist.tile([128, 3], F32, name="bias3", tag="bias3")
        b1t = persist.tile([128, DFF // 128], F32, name="b1t", tag="b1t")
        b2t = persist.tile([128, CT], F32, name="b2t", tag="b2t")
        lng = {}
        for nm, src in (("g1", g1v), ("be1", be1v), ("g2", g2v),
                        ("be2", be2v)):
            lng[nm] = persist.tile([128, CT], F32, name=nm, tag=nm)
            nc.sync.dma_start(lng[nm][:], src[:])
        nc.sync.dma_start(bias3[:], bqkv[:])
        nc.sync.dma_start(b1t[:], b1[:])
        nc.sync.dma_start(b2t[:], b2x2[:])

        nc.vector.memset(onesf[:], 1.0)
        nc.vector.tensor_copy(ones128[:], onesf[:])
        nc.gpsimd.memset(ident[:], 0.0)
        nc.gpsimd.affine_select(
            out=ident[:], in_=ident[:], compare_op=mybir.AluOpType.not_equal,
            fill=1.0, base=0, pattern=[[-1, 128]], channel_multiplier=1)

        # causal mask family: mbig[p, j] = 1 iff p <= j - 384, so that
        # mbig[:, 384+r : 896+r] is the 0/1 mask "keep k<=q" for a
        # diagonal block with relative offset r = qc*512 - kt*128.
        mbig = None
        if causal:
            mbig = persist.tile([128, 896], BF16, name="mbig", tag="mbig")
            nc.vector.memset(mbig[:], 1.0)
            nc.gpsimd.affine_select(
                out=mbig[:], in_=mbig[:], compare_op=mybir.AluOpType.is_ge,
                fill=0.0, base=-384, pattern=[[1, 896]],
                channel_multiplier=-1)

        # residual x slice, FFN1-weight preload (reused across reps);
        # grouped into few big tiles so each loads with one DMA (HWDGE
        # descriptor generation is a serial resource).
        xres_sb = [persist.tile([128, 4 * TSL], F32, name=f"xr{g}",
                                tag=f"xr{g}") for g in range(2)]
        wearly = ctx.enter_context(tc.tile_pool(name="wearly", bufs=1))
        # first half of W1 preloaded; second half streamed during FFN1
        w1pre = [wearly.tile([128, DFF // 2], BF16, name=f"w1p_{kt}",
                             tag=f"w1p_{kt}") for kt in range(CT)]

        a2a_in = [dram.tile([NCORES, 64, TSL], BF16, name=f"a2ai{i}",
                            tag=f"a2ai{i}") for i in range(2)]
        a2a_out = [dram.tile([NCORES, 64, TSL], BF16, name=f"a2ao{i}",
                             tag=f"a2ao{i}") for i in range(2)]

        x1f = [persist.tile([128, TSL], F32, name=f"x1f{i}", tag=f"x1f{i}")
               for i in range(CT)]
        x1b = [persist.tile([128, TSL], BF16, name=f"x1b{i}", tag=f"x1b{i}")
               for i in range(CT)]

        def layer_norm(zf, gt, bt, dst_f32, dst_bf16, pools, col=None,
                       out_dram=None):
            ps_ln, lnp = pools
            if col is None:
                col = slice(0, TSL)
            W = col.stop - col.start
            zf = [z[:, col] for z in zf]
            dst_f32 = [t[:, col] for t in dst_f32]
            if dst_bf16 is not None:
                dst_bf16 = [t[:, col] for t in dst_bf16]
            epst = lnp.tile([128, 1], F32, name="epst", tag="epst")
            nc.vector.memset(epst[:], EPS)
            sum_ps = ps_ln.tile([128, W], F32, name="sum_ps", tag="sum_ps")
            for i in range(CT):
                nc.tensor.matmul(sum_ps[:], ones128[:], zf[i][:],
                                 start=(i == 0), stop=(i == CT - 1))
            sq_ps = ps_ln.tile([128, W], F32, name="sq_ps", tag="sq_ps")
            for i in range(CT):
                zsq = lnp.tile([128, W], F32R, name="zsq", tag="zsq")
                nc.scalar.square(zsq[:], zf[i][:])
                nc.tensor.matmul(sq_ps[:], ones128[:], zsq[:],
                                 start=(i == 0), stop=(i == CT - 1))
            mu = lnp.tile([128, W], F32, name="mu", tag="mu")
            nc.vector.tensor_scalar_mul(mu[:], sum_ps[:], 1.0 / C)
            musq = lnp.tile([128, W], F32, name="musq", tag="musq")
            nc.vector.tensor_mul(musq[:], mu[:], mu[:])
            var = lnp.tile([128, W], F32, name="var", tag="var")
            nc.vector.scalar_tensor_tensor(
                var[:], sq_ps[:], 1.0 / C, musq[:],
                op0=mybir.AluOpType.mult, op1=mybir.AluOpType.subtract)
            std = lnp.tile([128, W], F32, name="std", tag="std")
            nc.scalar.activation(std[:], var[:],
                                 mybir.ActivationFunctionType.Sqrt,
                                 bias=epst[:])
            rstd = lnp.tile([128, W], F32, name="rstd", tag="rstd")
            nc.vector.reciprocal(rstd[:], std[:])
            for i in range(CT):
                t = lnp.tile([128, W], F32, name="lnt", tag="lnt")
                nc.vector.tensor_sub(t[:], zf[i][:], mu[:])
                t2 = lnp.tile([128, W], F32, name="lnt2", tag="lnt2")
                nc.vector.tensor_mul(t2[:], t[:], rstd[:])
                nc.scalar.activation(dst_f32[i][:], t2[:],
                                     mybir.ActivationFunctionType.Identity,
                                     bias=bt[:, i:i + 1], scale=gt[:, i:i + 1])
                if dst_bf16 is not None:
                    nc.vector.tensor_copy(dst_bf16[i][:], dst_f32[i][:])
                if out_dram is not None:
                    nc.sync.dma_start(out_dram[i][:, col], dst_f32[i][:])

        def one_rep(rep):
            # z1 / attention-gather tiles span phases 2-3
            zstack = ExitStack()
            zpool = zstack.enter_context(tc.tile_pool(name=f"zp{rep}",
                                                      bufs=1))
            # att_all rows 0:64 = head lh=0 of each pair, 64:128 = lh=1;
            # columns grouped by channel-pair i (= a2a chunk).
            att_all = zpool.tile([128, CT * TSL], BF16, name="att_all",
                                 tag="att_all")
            z1 = [zpool.tile([128, TSL], F32R, name=f"z1{i}",
                             tag=f"z1{i}") for i in range(CT)]

            def gather_half(lh):
                # load this head-half's resharded chunks and fold them into
                # z1 rows [lh*64:(lh+1)*64] (z1 = xres + 2*attn)
                rs = slice(lh * 64, (lh + 1) * 64)
                for i in range(CT):
                    nc.sync.dma_start(
                        att_all[rs, i * TSL:(i + 1) * TSL],
                        a2a_out[lh][i])
                for i in range(CT):
                    eng = nc.vector  # TensorScalarPtr unsupported on Pool
                    xrs = xres_sb[i // 4][rs,
                                          (i % 4) * TSL:(i % 4 + 1) * TSL]
                    eng.scalar_tensor_tensor(
                        z1[i][rs, :], att_all[rs, i * TSL:(i + 1) * TSL],
                        2.0, xrs, op0=mybir.AluOpType.mult,
                        op1=mybir.AluOpType.add)

            # ---- phases 1+2: QKV projections + attention ----
            with ExitStack() as c12:
                cattn = c12.enter_context(tc.tile_pool(name="cattn", bufs=1))
                # zero-padded per-head q (full-K scores matmuls -> FWL fast)
                qp = [cattn.tile([128, TOK], BF16, name=f"qp{h}",
                                 tag=f"qp{h}") for h in range(2)]
                kT = cattn.tile([128, TOK], BF16, name="kT", tag="kT")
                # token-major v tiles; cols h*65..h*65+64 = [v_h | ones],
                # cols 130..200 zero padding so lhsT can always be 128 wide
                vtok = [cattn.tile([128, 200], BF16, name=f"vtok{i}",
                                   tag=f"vtok{i}")
                        for i in range(TOK // 128)]

                with ExitStack() as c1:
                    xchp = c1.enter_context(tc.tile_pool(name="xchp",
                                                         bufs=1))
                    ps = c1.enter_context(tc.tile_pool(name="ps_qkv", bufs=4,
                                                       space="PSUM"))
                    pst = c1.enter_context(tc.tile_pool(name="ps_tr", bufs=2,
                                                        space="PSUM"))
                    wpool = c1.enter_context(tc.tile_pool(name="wqkv",
                                                          bufs=1))
                    vTf = c1.enter_context(tc.tile_pool(name="vTf", bufs=1))
                    vT = vTf.tile([128, TOK], BF16, name="vT", tag="vT")

                    # chunk-0 x first, then weights (one strided DMA per
                    # weight), then the remaining x chunks: minimizes time
                    # to the first matmul.
                    xc0 = xchp.tile([128, 4096], BF16, name="xc", tag="xc",
                                    bufs=3)
                    for i in range(CT):
                        nc.sync.dma_start(
                            xc0[:, i * 512:(i + 1) * 512],
                            xT[i, :, 0:512])
                    wts = []
                    for wi, wdram in enumerate((wq, wk, wv)):
                        wt = wpool.tile([128, C], BF16, name=f"w{wi}",
                                        tag=f"w{wi}")
                        for kt in range(CT):
                            nc.sync.dma_start(
                                wt[:, kt * 128:(kt + 1) * 128], wdram[kt])
                        wts.append(wt)

                    nc.vector.memset(qp[0][64:128, :], 0.0)
                    nc.vector.memset(qp[1][0:64, :], 0.0)

                    for ch in range(NCH):
                        cs = slice(ch * 512, (ch + 1) * 512)
                        if ch == 0:
                            xc = xc0
                        else:
                            xc = xchp.tile([128, 4096], BF16, name="xc",
                                           tag="xc", bufs=3)
                            for i in range(CT):
                                nc.sync.dma_start(
                                    xc[:, i * 512:(i + 1) * 512],
                                    xT[i, :, cs])
                        for wi, brow in ((0, 0), (1, 1), (2, 2)):
                            wt = wts[wi]
                            p = ps.tile([128, 512], F32)
                            for kt in range(CT):
                                nc.tensor.matmul(
                                    p[:], wt[:, kt * 128:(kt + 1) * 128],
                                    xc[:, kt * 512:(kt + 1) * 512],
                                    start=(kt == 0), stop=(kt == CT - 1))
                            if wi == 0:  # q: split heads into padded tiles
                                nc.vector.tensor_scalar_add(
                                    qp[0][0:64, cs], p[0:64, :],
                                    bias3[0:64, 0:1])
                                nc.vector.tensor_scalar_add(
                                    qp[1][64:128, cs], p[64:128, :],
                                    bias3[64:128, 0:1])
                            else:
                                dst = kT if wi == 1 else vT
                                nc.vector.tensor_scalar_add(
                                    dst[:, cs], p[:],
                                    bias3[:, brow:brow + 1])
                        # transpose this chunk's v to token-major tiles
                        # (vtok assembly on the otherwise-idle Pool engine)
                        for j in range(ch * 4, ch * 4 + 4):
                            pt = pst.tile([128, 128], BF16)
                            nc.tensor.matmul(pt[:],
                                             vT[:, j * 128:(j + 1) * 128],
                                             ident[:], is_transpose=True,
                                             start=True, stop=True)
                            nc.gpsimd.memset(vtok[j][:, 130:200], 0.0)
                            for lh in range(2):
                                # DVE: gpsimd cannot read PSUM (pt)
                                nc.vector.tensor_copy(
                                    vtok[j][:, lh * 65:lh * 65 + 64],
                                    pt[:, lh * 64:(lh + 1) * 64])
                            nc.gpsimd.memset(vtok[j][:, 64:65], 1.0)
                            nc.gpsimd.memset(vtok[j][:, 129:130], 1.0)

                # ---- phase 2: attention (scores transposed [k, q]) ----
                with ExitStack() as c2:
                    ps_sc = c2.enter_context(tc.tile_pool(name="ps_sc",
                                                          bufs=3,
                                                          space="PSUM"))
                    ps_pv = c2.enter_context(tc.tile_pool(name="ps_pv",
                                                          bufs=3,
                                                          space="PSUM"))
                    ps_bc = c2.enter_context(tc.tile_pool(name="ps_bc",
                                                          bufs=2,
                                                          space="PSUM"))
                    ptp = c2.enter_context(tc.tile_pool(name="ptp", bufs=4))
                    stp = c2.enter_context(tc.tile_pool(name="stage",
                                                        bufs=1))
                    mkp = c2.enter_context(tc.tile_pool(name="maskp",
                                                        bufs=3))

                    # prefetch post-attention tensors now: the DMA engines
                    # are idle through the whole attention phase.
                    for i in range(CT):
                        nc.sync.dma_start(
                            xres_sb[i // 4][:,
                                            (i % 4) * TSL:(i % 4 + 1) * TSL],
                            xres[i])
                    for kt in range(CT):
                        nc.sync.dma_start(
                            w1pre[kt][:],
                            w1[kt * 128:(kt + 1) * 128, 0:DFF // 2])

                    for lh in range(2):
                        for b in range(B):
                            for qc in range(NQC):
                                d = b * NQC + qc    # dest core / token chunk
                                q0 = b * T + qc * 512
                                nkt = (qc + 1) * 4 if causal else T // 128
                                pv = ps_pv.tile([128, TSL], F32)
                                for kt in range(nkt):
                                    kc = b * T + kt * 128
                                    sc = ps_sc.tile([128, 512], F32)
                                    nc.tensor.matmul(
                                        sc[:], kT[:, kc:kc + 128],
                                        qp[lh][:, q0:q0 + 512],
                                        start=True, stop=True)
                                    if masked:
                                        mkt = mkp.tile([128, 512], BF16)
                                        nc.sync.dma_start(
                                            mkt[:],
                                            amask[kt, :,
                                                  qc * 512:(qc + 1) * 512])
                                        nc.vector.tensor_add(sc[:], sc[:],
                                                             mkt[:])
                                    pt = ptp.tile([128, 512], BF16)
                                    nc.scalar.activation(
                                        pt[:], sc[:],
                                        mybir.ActivationFunctionType.Exp)
                                    if causal and kt >= 4 * qc:
                                        # zero entries with k > q via a
                                        # precomputed 0/1 mask slice (DVE)
                                        r = qc * 512 - kt * 128
                                        nc.vector.tensor_mul(
                                            pt[:], pt[:],
                                            mbig[:, 384 + r:896 + r])
                                    nc.tensor.matmul(
                                        pv[:], vtok[(b * T) // 128 + kt]
                                        [:, lh * 65:lh * 65 + 128], pt[:],
                                        start=(kt == 0),
                                        stop=(kt == nkt - 1))
                                # normalize by the denominator row (64) and
                                # ship; all hidden under attention compute
                                rec1 = stp.tile([1, TSL], F32R, name="rec1",
                                                tag="rec1", bufs=2)
                                with nc.allow_low_precision(
                                        reason="f32r recip of softmax denom"):
                                    nc.vector.reciprocal(rec1[:],
                                                         pv[64:65, :])
                                bc = ps_bc.tile([64, TSL], F32, name="bc",
                                                tag="bc")
                                nc.tensor.matmul(bc[:], ones128[0:1, 0:64],
                                                 rec1[:],
                                                 start=True, stop=True)
                                st = stp.tile([64, TSL], BF16, name="st",
                                              tag="st", bufs=3)
                                # copy then multiply in place: a DVE op may
                                # read at most one PSUM operand
                                nc.vector.tensor_copy(st[:], pv[0:64, :])
                                nc.vector.tensor_mul(st[:], st[:], bc[:])
                                nc.gpsimd.dma_start(a2a_in[lh][d], st[:])
                        if sim:
                            nc.sync.dma_start(a2a_out[lh][:], a2a_in[lh][:])
                        else:
                            nc.gpsimd.collective_compute(
                                "AllToAll", mybir.AluOpType.bypass,
                                replica_groups=[list(range(NCORES))],
                                ins=[a2a_in[lh].opt()],
                                outs=[a2a_out[lh].opt()])
                        gather_half(lh)

            # ---- phase 3: LN1 (z1 already assembled by gather_half) ----
            with ExitStack() as c3:
                ps_ln = c3.enter_context(tc.tile_pool(name="ps_ln", bufs=2,
                                                      space="PSUM"))
                lnp = c3.enter_context(tc.tile_pool(name="lnp", bufs=2))
                for hf in range(2):
                    layer_norm(z1, lng["g1"], lng["be1"], x1f, x1b,
                               (ps_ln, lnp),
                               col=slice(hf * (TSL // 2),
                                         (hf + 1) * (TSL // 2)))
            zstack.close()

            # ---- phase 4: FFN (token slice, full dff) ----
            with ExitStack() as c4:
                ps_f = c4.enter_context(tc.tile_pool(name="ps_f", bufs=4,
                                                     space="PSUM"))
                wp = c4.enter_context(tc.tile_pool(name="wp", bufs=3))
                hp = c4.enter_context(tc.tile_pool(name="hp", bufs=1))
                w1s = c4.enter_context(tc.tile_pool(name="w1s", bufs=1))
                h = [hp.tile([128, TSL], BF16, name=f"h{m}", tag=f"h{m}")
                     for m in range(DFF // 128)]
                # stream W1's second half now (DMA engines are idle until
                # the W2 stream starts)
                w1str = [w1s.tile([128, DFF // 2], BF16, name=f"w1s_{kt}",
                                  tag=f"w1s_{kt}") for kt in range(CT)]
                for kt in range(CT):
                    nc.sync.dma_start(
                        w1str[kt][:],
                        w1[kt * 128:(kt + 1) * 128, DFF // 2:])

                for mg in range(DFF // 512):  # 8 groups of 4 dff tiles
                    for mi in range(4):
                        m = mg * 4 + mi
                        if mg < 4:
                            w1t, ms = w1pre, slice(m * 128, (m + 1) * 128)
                        else:
                            w1t = w1str
                            ms = slice((m - 16) * 128, (m - 15) * 128)
                        for hf in range(2):
                            cs = slice(hf * (TSL // 2),
                                       (hf + 1) * (TSL // 2))
                            p = ps_f.tile([128, TSL // 2], F32, name="p",
                                          tag="p", bufs=2)
                            for kt in range(CT):
                                nc.tensor.matmul(
                                    p[:], w1t[kt][:, ms],
                                    x1b[kt][:, cs],
                                    start=(kt == 0), stop=(kt == CT - 1))
                            nc.vector.tensor_scalar(
                                h[m][:, cs], p[:], b1t[:, m:m + 1], 0.0,
                                mybir.AluOpType.add, mybir.AluOpType.max)

                # FFN2 + residual into z2
                z2 = [hp.tile([128, TSL], F32R, name=f"z2{i}", tag=f"z2{i}")
                      for i in range(CT)]
                NKT2 = DFF // 128
                for cg in range(C // 512):  # 2 groups of 4 C tiles
                    pcs = [ps_f.tile([128, TSL], F32, name=f"pc{ci}",
                                     tag=f"pc{ci}", bufs=1)
                           for ci in range(4)]
                    for kt in range(NKT2):
                        w2t = wp.tile([128, 512], BF16, name="w2t",
                                      tag="w2t", bufs=12)
                        nc.sync.dma_start(
                            w2t[:], w2[kt * 128:(kt + 1) * 128,
                                       cg * 512:(cg + 1) * 512])
                        for ci in range(4):
                            nc.tensor.matmul(
                                pcs[ci][:], w2t[:, ci * 128:(ci + 1) * 128],
                                h[kt][:], start=(kt == 0),
                                stop=(kt == NKT2 - 1))
                    for ci in range(4):
                        i = cg * 4 + ci
                        t = hp.tile([128, TSL], F32, name="ffo", tag="ffo")
                        nc.scalar.activation(
                            t[:], pcs[ci][:],
                            mybir.ActivationFunctionType.Identity,
                            bias=b2t[:, i:i + 1], scale=2.0)
                        nc.vector.tensor_add(z2[i][:], t[:], x1f[i][:])

                # ---- phase 5: LN2 + output ----
                ps_ln2 = c4.enter_context(tc.tile_pool(name="ps_ln2",
                                                       bufs=1, space="PSUM"))
                lnp2 = c4.enter_context(tc.tile_pool(name="lnp2", bufs=2))
                outf = [hp.tile([128, TSL], F32, name=f"of{i}",
                                tag=f"of{i}") for i in range(CT)]
                for hf in range(2):
                    layer_norm(z2, lng["g2"], lng["be2"], outf, None,
                               (ps_ln2, lnp2), out_dram=out,
                               col=slice(hf * (TSL // 2),
                                         (hf + 1) * (TSL // 2)))

        for rep in range(reps):
            one_rep(rep)


def _build(causal: bool, masked: bool, sim: bool = False, reps: int = 1):
    key = (causal, masked, sim, reps)
    if key in _CACHE:
        return _CACHE[key]
    nc = bacc.Bacc("TRN2", target_bir_lowering=False, debug=False,
                   num_devices=NCORES)
    _emit(nc, causal, masked, sim=sim, reps=reps)
    nc.compile()
    _CACHE[key] = nc
    return nc


def kernel(x, attention_mask, Wq, bq, Wk, bk, Wv, bv, W1, b1, W2, b2,
           g1, be1, g2, be2):
    global LAST_EXEC_NS
    f32 = np.float32
    x = np.asarray(x, f32).reshape(TOK, C)
    xT = np.ascontiguousarray(x.T)                      # [C, TOK]
    mask = np.asarray(attention_mask).reshape(T, T)

    causal = bool(np.array_equal(
        mask != 0, np.tril(np.ones((T, T), dtype=bool))))
    masked = (not causal) and not bool((mask != 0).all())

    Wq = np.asarray(Wq, f32); Wk = np.asarray(Wk, f32)
    Wv = np.asarray(Wv, f32); W1 = np.asarray(W1, f32)
    W2 = np.asarray(W2, f32)
    bq = np.asarray(bq, f32); bk = np.asarray(bk, f32)
    bv = np.asarray(bv, f32); b1 = np.asarray(b1, f32)
    b2 = np.asarray(b2, f32)
    scale = 1.0 / math.sqrt(DK)

    shared = dict(
        xT=xT.reshape(CT, 128, TOK).astype(BF),
        w1=W1.astype(BF),
        b1=np.ascontiguousarray(b1.reshape(DFF // 128, 128).T),
        w2=W2.astype(BF),
        b2x2=np.ascontiguousarray((2.0 * b2).reshape(CT, 128).T),
        g1v=np.ascontiguousarray(np.asarray(g1, f32).reshape(CT, 128).T),
        be1v=np.ascontiguousarray(np.asarray(be1, f32).reshape(CT, 128).T),
        g2v=np.ascontiguousarray(np.asarray(g2, f32).reshape(CT, 128).T),
        be2v=np.ascontiguousarray(np.asarray(be2, f32).reshape(CT, 128).T),
    )
    if masked:
        add = np.where(mask != 0, 0.0, -30000.0).astype(f32)
        shared["amask"] = np.ascontiguousarray(add.T).reshape(
            T // 128, 128, T).astype(BF)

    in_maps = []
    for c in range(NCORES):
        hs = slice(c * 128, (c + 1) * 128)
        m = dict(shared)
        m["xres"] = np.ascontiguousarray(
            xT[:, c * TSL:(c + 1) * TSL]).reshape(CT, 128, TSL)
        m["wq"] = np.ascontiguousarray(
            Wq[:, hs] * scale).reshape(CT, 128, 128).astype(BF)
        m["wk"] = np.ascontiguousarray(Wk[:, hs]).reshape(
            CT, 128, 128).astype(BF)
        m["wv"] = np.ascontiguousarray(Wv[:, hs]).reshape(
            CT, 128, 128).astype(BF)
        m["bqkv"] = np.ascontiguousarray(
            np.stack([bq[hs] * scale, bk[hs], bv[hs]], axis=1)).astype(f32)
        in_maps.append(m)

    reps = int(os.environ.get("KERNEL_REPS", "1"))
    nc = _build(causal, masked, reps=reps)
    res = run_bass_kernel_spmd(nc, in_maps, list(range(NCORES)))
    LAST_EXEC_NS = res.exec_time_ns

    outT = np.concatenate(
        [res.results[c]["out"].reshape(C, TSL) for c in range(NCORES)],
        axis=1)                                          # [C, TOK]
    return np.ascontiguousarray(outT.T).reshape(B, T, C).astype(f32)



# revision 25
# speedup vs baseline: 1.0668x; 1.0668x over previous
"""Trainium2 Bass kernel for a dense transformer attention layer.

Computes, for x:[2,2048,1024] (B=2, T=2048, D=1024, H=16 heads, dk=64,
dff=4096):
    q,k,v = split_heads(x@Wq+bq), ...     (per-head dims 64)
    attn  = softmax(causal_mask(q k^T / 8)) v
    x1    = LN(x + 2*attn; g1, be1)
    out   = LN(x1 + 2*(relu(x1@W1+b1)@W2+b2); g2, be2)

Distribution over 8 NeuronCores:
  - QKV + attention: head-parallel (2 heads per core), all tokens.
    Identical causal loop structure on every core (SPMD-safe).
  - Two AllToAlls (one per local head) reshard attention output from
    head-major to token-major; the first overlaps with the second
    head's attention compute. Payload rows carry the unnormalized
    attention plus the softmax denominator row; normalization happens
    after the collective, fused into the residual-add.
  - LN1 + FFN + LN2: token-parallel (512 tokens per core, full dff).

On-chip layout is feature-major ([channels, tokens]) everywhere, so no
activation transposes are needed for matmuls; attention scores are
computed transposed ([k, q]) so the softmax denominator falls out of the
PV matmul via an extra ones-column in the token-major V tiles.
x streams in 512-token chunks so QKV matmuls start ~5us into the
kernel instead of waiting for the full 8MB activation load.
"""
import os
import math
from contextlib import ExitStack

import numpy as np

import concourse.bass as bass
import concourse.tile as tile
from concourse import bacc, mybir
from concourse.bass_utils import run_bass_kernel_spmd

F32 = mybir.dt.float32
F32R = mybir.dt.float32r
BF16 = mybir.dt.bfloat16
F8 = mybir.dt.float8e4
BF = np.dtype("bfloat16")

NCORES = 8
B, T, C, DK, H, DFF = 2, 2048, 1024, 64, 16, 4096
TOK = B * T            # 4096 tokens
TSL = TOK // NCORES    # 512 tokens per core (post-attention shard)
CT = C // 128          # 8 channel tiles
NCH = TOK // 512       # 8 token chunks
EPS = 1e-5

_CACHE = {}
LAST_EXEC_NS = None


def _emit(nc, causal: bool, masked: bool, sim: bool = False, reps: int = 1):
    """Emit the SPMD program. causal: skip/selective-mask causal blocks.
    masked: add a generic additive mask input (maskT, [k,q] layout).
    sim: replace collectives with local DRAM copies (TimelineSim has no
    collective cost model). reps: run the whole layer this many times
    (benchmark amortization; output is rewritten identically)."""
    dt_in = nc.dram_tensor
    xT = dt_in("xT", [CT, 128, TOK], BF16, kind="ExternalInput").ap()
    xres = dt_in("xres", [CT, 128, TSL], F32, kind="ExternalInput").ap()
    wq = dt_in("wq", [CT, 128, 128], BF16, kind="ExternalInput").ap()
    wk = dt_in("wk", [CT, 128, 128], BF16, kind="ExternalInput").ap()
    wv = dt_in("wv", [CT, 128, 128], BF16, kind="ExternalInput").ap()
    bqkv = dt_in("bqkv", [128, 3], F32, kind="ExternalInput").ap()
    w1 = dt_in("w1", [C, DFF], BF16, kind="ExternalInput").ap()
    b1 = dt_in("b1", [128, DFF // 128], F32, kind="ExternalInput").ap()
    w2 = dt_in("w2", [DFF, C], BF16, kind="ExternalInput").ap()
    b2x2 = dt_in("b2x2", [128, CT], F32, kind="ExternalInput").ap()
    g1v = dt_in("g1v", [128, CT], F32, kind="ExternalInput").ap()
    be1v = dt_in("be1v", [128, CT], F32, kind="ExternalInput").ap()
    g2v = dt_in("g2v", [128, CT], F32, kind="ExternalInput").ap()
    be2v = dt_in("be2v", [128, CT], F32, kind="ExternalInput").ap()
    amask = None
    if masked:
        # additive mask, transposed: amask[kt][k 128, q 2048] (bf16, 0/-30000)
        amask = dt_in("amask", [T // 128, 128, T], BF16,
                      kind="ExternalInput").ap()
    out = dt_in("out", [CT, 128, TSL], F32, kind="ExternalOutput").ap()

    NQC = T // 512  # 4 q-chunks of 512 per batch

    with tile.TileContext(nc) as tc, ExitStack() as ctx:
        persist = ctx.enter_context(tc.tile_pool(name="persist", bufs=1))
        dram = ctx.enter_context(tc.tile_pool(name="dram", bufs=1,
                                              space="DRAM"))

        # ---- persistent SBUF tensors (small; live whole kernel) ----
        ident = persist.tile([128, 128], BF16, name="ident", tag="ident")
        ones128 = persist.tile([128, 128], F32R, name="ones128", tag="ones128")
        onesf = persist.tile([128, 128], F32, name="onesf", tag="onesf")
        bias3 = persist.tile([128, 3], F32, name="bias3", tag="bias3")
        b1t = persist.tile([128, DFF // 128], F32, name="b1t", tag="b1t")
        b2t = persist.tile([128, CT], F32, name="b2t", tag="b2t")
        lng = {}
        for nm, src in (("g1", g1v), ("be1", be1v), ("g2", g2v),
                        ("be2", be2v)):
            lng[nm] = persist.tile([128, CT], F32, name=nm, tag=nm)
            nc.sync.dma_start(lng[nm][:], src[:])
        nc.sync.dma_start(bias3[:], bqkv[:])
        nc.sync.dma_start(b1t[:], b1[:])
        nc.sync.dma_start(b2t[:], b2x2[:])

        nc.vector.memset(onesf[:], 1.0)
        nc.vector.tensor_copy(ones128[:], onesf[:])
        nc.gpsimd.memset(ident[:], 0.0)
        nc.gpsimd.affine_select(
            out=ident[:], in_=ident[:], compare_op=mybir.AluOpType.not_equal,
            fill=1.0, base=0, pattern=[[-1, 128]], channel_multiplier=1)

        # causal mask family: mbig[p, j] = 1 iff p <= j - 384, so that
        # mbig[:, 384+r : 896+r] is the 0/1 mask "keep k<=q" for a
        # diagonal block with relative offset r = qc*512 - kt*128.
        mbig = None
        if causal:
            mbig = persist.tile([128, 896], BF16, name="mbig", tag="mbig")
            nc.vector.memset(mbig[:], 1.0)
            nc.gpsimd.affine_select(
                out=mbig[:], in_=mbig[:], compare_op=mybir.AluOpType.is_ge,
                fill=0.0, base=-384, pattern=[[1, 896]],
                channel_multiplier=-1)

        # residual x slice, FFN1-weight preload (reused across reps);
        # grouped into few big tiles so each loads with one DMA (HWDGE
        # descriptor generation is a serial resource).
        xres_sb = [persist.tile([128, 4 * TSL], F32, name=f"xr{g}",
                                tag=f"xr{g}") for g in range(2)]
        wearly = ctx.enter_context(tc.tile_pool(name="wearly", bufs=1))
        # first half of W1 preloaded; second half streamed during FFN1
        w1pre = [wearly.tile([128, DFF // 2], BF16, name=f"w1p_{kt}",
                             tag=f"w1p_{kt}") for kt in range(CT)]

        # 65 rows per chunk: 64 unnormalized attention rows + the
        # softmax denominator row; normalization happens receiver-side,
        # batched (one [8,512] reciprocal per head-half instead of 16
        # serial [1,512] reciprocals on the sender's critical path).
        dramden = dram.tile([NCORES, TSL], F32, name="dramden",
                            tag="dramden")
        a2a_in = [dram.tile([NCORES, 65, TSL], BF16, name=f"a2ai{i}",
                            tag=f"a2ai{i}") for i in range(2)]
        a2a_out = [dram.tile([NCORES, 65, TSL], BF16, name=f"a2ao{i}",
                             tag=f"a2ao{i}") for i in range(2)]

        x1f = [persist.tile([128, TSL], F32, name=f"x1f{i}", tag=f"x1f{i}")
               for i in range(CT)]
        x1b = [persist.tile([128, TSL], BF16, name=f"x1b{i}", tag=f"x1b{i}")
               for i in range(CT)]

        def layer_norm(zf, gt, bt, dst_f32, dst_bf16, pools, col=None,
                       out_dram=None):
            ps_ln, lnp = pools
            if col is None:
                col = slice(0, TSL)
            W = col.stop - col.start
            zf = [z[:, col] for z in zf]
            dst_f32 = [t[:, col] for t in dst_f32]
            if dst_bf16 is not None:
                dst_bf16 = [t[:, col] for t in dst_bf16]
            epst = lnp.tile([128, 1], F32, name="epst", tag="epst")
            nc.vector.memset(epst[:], EPS)
            sum_ps = ps_ln.tile([128, W], F32, name="sum_ps", tag="sum_ps")
            for i in range(CT):
                nc.tensor.matmul(sum_ps[:], ones128[:], zf[i][:],
                                 start=(i == 0), stop=(i == CT - 1))
            sq_ps = ps_ln.tile([128, W], F32, name="sq_ps", tag="sq_ps")
            for i in range(CT):
                zsq = lnp.tile([128, W], F32R, name="zsq", tag="zsq")
                nc.scalar.square(zsq[:], zf[i][:])
                nc.tensor.matmul(sq_ps[:], ones128[:], zsq[:],
                                 start=(i == 0), stop=(i == CT - 1))
            mu = lnp.tile([128, W], F32, name="mu", tag="mu")
            nc.vector.tensor_scalar_mul(mu[:], sum_ps[:], 1.0 / C)
            musq = lnp.tile([128, W], F32, name="musq", tag="musq")
            nc.vector.tensor_mul(musq[:], mu[:], mu[:])
            var = lnp.tile([128, W], F32, name="var", tag="var")
            nc.vector.scalar_tensor_tensor(
                var[:], sq_ps[:], 1.0 / C, musq[:],
                op0=mybir.AluOpType.mult, op1=mybir.AluOpType.subtract)
            std = lnp.tile([128, W], F32, name="std", tag="std")
            nc.scalar.activation(std[:], var[:],
                                 mybir.ActivationFunctionType.Sqrt,
                                 bias=epst[:])
            rstd = lnp.tile([128, W], F32, name="rstd", tag="rstd")
            nc.vector.reciprocal(rstd[:], std[:])
            for i in range(CT):
                t = lnp.tile([128, W], F32, name="lnt", tag="lnt")
                nc.vector.tensor_sub(t[:], zf[i][:], mu[:])
                t2 = lnp.tile([128, W], F32, name="lnt2", tag="lnt2")
                nc.vector.tensor_mul(t2[:], t[:], rstd[:])
                nc.scalar.activation(dst_f32[i][:], t2[:],
                                     mybir.ActivationFunctionType.Identity,
                                     bias=bt[:, i:i + 1], scale=gt[:, i:i + 1])
                if dst_bf16 is not None:
                    nc.vector.tensor_copy(dst_bf16[i][:], dst_f32[i][:])
                if out_dram is not None:
                    nc.sync.dma_start(out_dram[i][:, col], dst_f32[i][:])

        def one_rep(rep):
            # z1 / attention-gather tiles span phases 2-3
            zstack = ExitStack()
            zpool = zstack.enter_context(tc.tile_pool(name=f"zp{rep}",
                                                      bufs=1))
            # att_all rows 0:64 = head lh=0 of each pair, 64:128 = lh=1;
            # columns grouped by channel-pair i (= a2a chunk).
            att_all = zpool.tile([128, CT * TSL], BF16, name="att_all",
                                 tag="att_all")
            z1 = [zpool.tile([128, TSL], F32R, name=f"z1{i}",
                             tag=f"z1{i}") for i in range(CT)]

            def gather_half(lh):
                # load this head-half's resharded chunks, normalize by the
                # shipped denominators, and fold into z1 rows
                # [lh*64:(lh+1)*64] (z1 = xres + 2*attn). One batched
                # reciprocal covers all 8 denominator rows (DVE reciprocal
                # cost is per-column). Every DVE op keeps in/out on the
                # same partitions (lanes are per-partition; sim does not
                # model this but hardware does):
                #  - lh=0 works on partitions 0:64; the broadcast row
                #    reaches partition 0 by DMA, then gpsimd
                #    partition_broadcast (base-0 only) fans it out.
                #  - lh=1 works on partitions 64:128; broadcast rows are
                #    written there directly by a zero-stride replication
                #    DMA from a DRAM staging row.
                rs = slice(lh * 64, (lh + 1) * 64)
                nc.sync.dma_start(
                    att_all[rs, :],
                    a2a_out[lh][:, 0:64, :].rearrange("i p c -> p i c"))
                dT = zpool.tile([128, TSL], BF16, name=f"dT{lh}",
                                tag=f"dT{lh}")
                nc.gpsimd.memset(dT[:], 1.0)
                nc.sync.dma_start(dT[0:8, :], a2a_out[lh][:, 64, :])
                rec = zpool.tile([128, TSL], F32, name=f"rec{lh}",
                                 tag=f"rec{lh}")
                nc.vector.reciprocal(rec[:], dT[:])
                if lh == 1:
                    nc.sync.dma_start(dramden[:], rec[0:8, :])
                for i in range(CT):
                    rbc = zpool.tile([128, TSL], F32, name="rbc", tag="rbc",
                                     bufs=2)
                    if lh == 0:
                        den = zpool.tile([1, TSL], F32, name="den",
                                         tag="den", bufs=4)
                        nc.sync.dma_start(den[:], rec[i:i + 1, :])
                        nc.gpsimd.partition_broadcast(rbc[0:64, :], den[:])
                    else:
                        nc.sync.dma_start(
                            rbc[64:128, :],
                            dramden[i:i + 1, :].broadcast_to([64, TSL]))
                    tt = zpool.tile([128, TSL], F32, name="tt", tag="tt",
                                    bufs=2)
                    nc.vector.tensor_mul(
                        tt[rs, :], att_all[rs, i * TSL:(i + 1) * TSL],
                        rbc[rs, :])
                    xrs = xres_sb[i // 4][rs,
                                          (i % 4) * TSL:(i % 4 + 1) * TSL]
                    nc.vector.scalar_tensor_tensor(
                        z1[i][rs, :], tt[rs, :], 2.0, xrs,
                        op0=mybir.AluOpType.mult, op1=mybir.AluOpType.add)

            # ---- phases 1+2: QKV projections + attention ----
            with ExitStack() as c12:
                cattn = c12.enter_context(tc.tile_pool(name="cattn", bufs=1))
                # zero-padded per-head q (full-K scores matmuls -> FWL fast)
                qp = [cattn.tile([128, TOK], BF16, name=f"qp{h}",
                                 tag=f"qp{h}") for h in range(2)]
                kT = cattn.tile([128, TOK], BF16, name="kT", tag="kT")
                # token-major v tiles; cols h*65..h*65+64 = [v_h | ones],
                # cols 130..200 zero padding so lhsT can always be 128 wide
                vtok = [cattn.tile([128, 200], BF16, name=f"vtok{i}",
                                   tag=f"vtok{i}")
                        for i in range(TOK // 128)]

                with ExitStack() as c1:
                    xchp = c1.enter_context(tc.tile_pool(name="xchp",
                                                         bufs=1))
                    ps = c1.enter_context(tc.tile_pool(name="ps_qkv", bufs=4,
                                                       space="PSUM"))
                    pst = c1.enter_context(tc.tile_pool(name="ps_tr", bufs=2,
                                                        space="PSUM"))
                    wpool = c1.enter_context(tc.tile_pool(name="wqkv",
                                                          bufs=1))
                    vTf = c1.enter_context(tc.tile_pool(name="vTf", bufs=1))
                    vT = vTf.tile([128, TOK], BF16, name="vT", tag="vT")

                    # chunk-0 x first, then weights (one strided DMA per
                    # weight), then the remaining x chunks: minimizes time
                    # to the first matmul.
                    xc0 = xchp.tile([128, 4096], BF16, name="xc", tag="xc",
                                    bufs=3)
                    nc.sync.dma_start(
                        xc0[:], xT[:, :, 0:512].rearrange("i p c -> p i c"))
                    wts = []
                    for wi, wdram in enumerate((wq, wk, wv)):
                        wt = wpool.tile([128, C], BF16, name=f"w{wi}",
                                        tag=f"w{wi}")
                        nc.sync.dma_start(
                            wt[:],
                            wdram[:].rearrange("i p c -> p i c"))
                        wts.append(wt)

                    nc.vector.memset(qp[0][64:128, :], 0.0)
                    nc.vector.memset(qp[1][0:64, :], 0.0)

                    for ch in range(NCH):
                        cs = slice(ch * 512, (ch + 1) * 512)
                        if ch == 0:
                            xc = xc0
                        else:
                            xc = xchp.tile([128, 4096], BF16, name="xc",
                                           tag="xc", bufs=3)
                            nc.sync.dma_start(
                                xc[:],
                                xT[:, :, cs].rearrange("i p c -> p i c"))
                        for wi, brow in ((0, 0), (1, 1), (2, 2)):
                            wt = wts[wi]
                            p = ps.tile([128, 512], F32)
                            for kt in range(CT):
                                nc.tensor.matmul(
                                    p[:], wt[:, kt * 128:(kt + 1) * 128],
                                    xc[:, kt * 512:(kt + 1) * 512],
                                    start=(kt == 0), stop=(kt == CT - 1))
                            if wi == 0:  # q: split heads into padded tiles
                                nc.vector.tensor_scalar_add(
                                    qp[0][0:64, cs], p[0:64, :],
                                    bias3[0:64, 0:1])
                                nc.vector.tensor_scalar_add(
                                    qp[1][64:128, cs], p[64:128, :],
                                    bias3[64:128, 0:1])
                            else:
                                dst = kT if wi == 1 else vT
                                nc.vector.tensor_scalar_add(
                                    dst[:, cs], p[:],
                                    bias3[:, brow:brow + 1])
                        # transpose this chunk's v to token-major tiles
                        # (vtok assembly on the otherwise-idle Pool engine)
                        for j in range(ch * 4, ch * 4 + 4):
                            pt = pst.tile([128, 128], BF16)
                            nc.tensor.matmul(pt[:],
                                             vT[:, j * 128:(j + 1) * 128],
                                             ident[:], is_transpose=True,
                                             start=True, stop=True)
                            nc.gpsimd.memset(vtok[j][:, 130:200], 0.0)
                            for lh in range(2):
                                # DVE: gpsimd cannot read PSUM (pt)
                                nc.vector.tensor_copy(
                                    vtok[j][:, lh * 65:lh * 65 + 64],
                                    pt[:, lh * 64:(lh + 1) * 64])
                            nc.gpsimd.memset(vtok[j][:, 64:65], 1.0)
                            nc.gpsimd.memset(vtok[j][:, 129:130], 1.0)

                # ---- phase 2: attention (scores transposed [k, q]) ----
                with ExitStack() as c2:
                    ps_sc = c2.enter_context(tc.tile_pool(name="ps_sc",
                                                          bufs=6,
                                                          space="PSUM"))
                    ps_pv = c2.enter_context(tc.tile_pool(name="ps_pv",
                                                          bufs=2,
                                                          space="PSUM"))
                    ptp = c2.enter_context(tc.tile_pool(name="ptp", bufs=4))
                    stp = c2.enter_context(tc.tile_pool(name="stage",
                                                        bufs=1))
                    mkp = c2.enter_context(tc.tile_pool(name="maskp",
                                                        bufs=3))

                    # prefetch post-attention tensors now: the DMA engines
                    # are idle through the whole attention phase.
                    for g in range(2):
                        nc.sync.dma_start(
                            xres_sb[g][:],
                            xres[g * 4:(g + 1) * 4].rearrange(
                                "i p c -> p i c"))
                    for kt in range(CT):
                        nc.sync.dma_start(
                            w1pre[kt][:],
                            w1[kt * 128:(kt + 1) * 128, 0:DFF // 2])

                    for lh in range(2):
                        for b in range(B):
                            for qc in range(NQC):
                                d = b * NQC + qc    # dest core / token chunk
                                q0 = b * T + qc * 512
                                nkt = (qc + 1) * 4 if causal else T // 128
                                pv = ps_pv.tile([128, TSL], F32)
                                # Diagonal causal tiles are narrowed to the
                                # columns with k <= q reachable (off = first
                                # valid q column in the 512-wide chunk):
                                # fewer PE cycles AND ~30% fewer ACT Exp
                                # elements than computing the full tile.
                                for kt in range(nkt):
                                    kc = b * T + kt * 128
                                    off = (max(0, (kt - 4 * qc) * 128)
                                           if causal else 0)
                                    sc = ps_sc.tile([128, 512], F32)
                                    nc.tensor.matmul(
                                        sc[:, off:512], kT[:, kc:kc + 128],
                                        qp[lh][:, q0 + off:q0 + 512],
                                        start=True, stop=True)
                                    if masked:
                                        mkt = mkp.tile([128, 512], BF16)
                                        nc.sync.dma_start(
                                            mkt[:],
                                            amask[kt, :,
                                                  qc * 512:(qc + 1) * 512])
                                        nc.vector.tensor_add(sc[:], sc[:],
                                                             mkt[:])
                                    pt = ptp.tile([128, 512], BF16)
                                    nc.scalar.activation(
                                        pt[:, off:512], sc[:, off:512],
                                        mybir.ActivationFunctionType.Exp)
                                    if causal and kt >= 4 * qc:
                                        # zero k > q inside the valid
                                        # region (0/1 mask slice, DVE)
                                        w = 512 - off
                                        nc.vector.tensor_mul(
                                            pt[:, off:512], pt[:, off:512],
                                            mbig[:, 384:384 + w])
                                    nc.tensor.matmul(
                                        pv[:, off:512],
                                        vtok[(b * T) // 128 + kt]
                                        [:, lh * 65:lh * 65 + 128],
                                        pt[:, off:512],
                                        start=(kt == 0),
                                        stop=(kt == nkt - 1))
                                # ship unnormalized rows + denominator
                                # row; one DVE cast, nothing else in the
                                # tail (no PE/GpSimd entanglement, pv
                                # bank freed immediately).
                                st = stp.tile([65, TSL], BF16, name="st",
                                              tag="st", bufs=3)
                                nc.vector.tensor_copy(st[:], pv[0:65, :])
                                nc.gpsimd.dma_start(a2a_in[lh][d], st[:])
                        if sim:
                            nc.sync.dma_start(a2a_out[lh][:], a2a_in[lh][:])
                        else:
                            nc.gpsimd.collective_compute(
                                "AllToAll", mybir.AluOpType.bypass,
                                replica_groups=[list(range(NCORES))],
                                ins=[a2a_in[lh].opt()],
                                outs=[a2a_out[lh].opt()])
                        if lh == 1:
                            # keep the PE busy while the second AllToAll is
                            # in flight: an idle gap >3.4us re-throttles the
                            # HAM clock gate to K=4/8 and LN1+FFN would then
                            # run at half clock for the next ~3.4us window.
                            nwarm = int(os.environ.get("KERNEL_NWARM", "0"))
                            if nwarm:
                                wdum = ps_pv.tile([128, TSL], F32,
                                                  name="wdum", tag="wdum",
                                                  bufs=1)
                                for _ in range(nwarm):
                                    nc.tensor.matmul(
                                        wdum[:], ident[:], qp[0][:, 0:512],
                                        start=True, stop=True)
                        gather_half(lh)

            # ---- phase 3: LN1 (z1 already assembled by gather_half) ----
            with ExitStack() as c3:
                ps_ln = c3.enter_context(tc.tile_pool(name="ps_ln", bufs=2,
                                                      space="PSUM"))
                lnp = c3.enter_context(tc.tile_pool(name="lnp", bufs=2))
                for hf in range(2):
                    layer_norm(z1, lng["g1"], lng["be1"], x1f, x1b,
                               (ps_ln, lnp),
                               col=slice(hf * (TSL // 2),
                                         (hf + 1) * (TSL // 2)))
            zstack.close()

            # ---- phase 4: FFN (token slice, full dff) ----
            with ExitStack() as c4:
                ps_f = c4.enter_context(tc.tile_pool(name="ps_f", bufs=4,
                                                     space="PSUM"))
                wp = c4.enter_context(tc.tile_pool(name="wp", bufs=3))
                hp = c4.enter_context(tc.tile_pool(name="hp", bufs=1))
                w1s = c4.enter_context(tc.tile_pool(name="w1s", bufs=1))
                h = [hp.tile([128, TSL], BF16, name=f"h{m}", tag=f"h{m}")
                     for m in range(DFF // 128)]
                # stream W1's second half now (DMA engines are idle until
                # the W2 stream starts)
                w1str = [w1s.tile([128, DFF // 2], BF16, name=f"w1s_{kt}",
                                  tag=f"w1s_{kt}") for kt in range(CT)]
                for kt in range(CT):
                    nc.sync.dma_start(
                        w1str[kt][:],
                        w1[kt * 128:(kt + 1) * 128, DFF // 2:])

                for mg in range(DFF // 512):  # 8 groups of 4 dff tiles
                    for mi in range(4):
                        m = mg * 4 + mi
                        if mg < 4:
                            w1t, ms = w1pre, slice(m * 128, (m + 1) * 128)
                        else:
                            w1t = w1str
                            ms = slice((m - 16) * 128, (m - 15) * 128)
                        for hf in range(2):
                            cs = slice(hf * (TSL // 2),
                                       (hf + 1) * (TSL // 2))
                            p = ps_f.tile([128, TSL // 2], F32, name="p",
                                          tag="p", bufs=2)
                            for kt in range(CT):
                                nc.tensor.matmul(
                                    p[:], w1t[kt][:, ms],
                                    x1b[kt][:, cs],
                                    start=(kt == 0), stop=(kt == CT - 1))
                            # relu(p + b1) on ACT (idle during FFN1; frees
                            # the DVE, which was co-critical with the PE)
                            nc.scalar.activation(
                                h[m][:, cs], p[:],
                                mybir.ActivationFunctionType.Relu,
                                bias=b1t[:, m:m + 1])

                # FFN2 + residual into z2
                z2 = [hp.tile([128, TSL], F32R, name=f"z2{i}", tag=f"z2{i}")
                      for i in range(CT)]
                NKT2 = DFF // 128
                for cg in range(C // 512):  # 2 groups of 4 C tiles
                    pcs = [ps_f.tile([128, TSL], F32, name=f"pc{ci}",
                                     tag=f"pc{ci}", bufs=1)
                           for ci in range(4)]
                    for kt in range(NKT2):
                        w2t = wp.tile([128, 512], BF16, name="w2t",
                                      tag="w2t", bufs=12)
                        nc.sync.dma_start(
                            w2t[:], w2[kt * 128:(kt + 1) * 128,
                                       cg * 512:(cg + 1) * 512])
                        for ci in range(4):
                            nc.tensor.matmul(
                                pcs[ci][:], w2t[:, ci * 128:(ci + 1) * 128],
                                h[kt][:], start=(kt == 0),
                                stop=(kt == NKT2 - 1))
                    for ci in range(4):
                        i = cg * 4 + ci
                        t = hp.tile([128, TSL], F32, name="ffo", tag="ffo")
                        nc.scalar.activation(
                            t[:], pcs[ci][:],
                            mybir.ActivationFunctionType.Identity,
                            bias=b2t[:, i:i + 1], scale=2.0)
                        nc.vector.tensor_add(z2[i][:], t[:], x1f[i][:])

                # ---- phase 5: LN2 + output ----
                ps_ln2 = c4.enter_context(tc.tile_pool(name="ps_ln2",
                                                       bufs=1, space="PSUM"))
                lnp2 = c4.enter_context(tc.tile_pool(name="lnp2", bufs=2))
                outf = [hp.tile([128, TSL], F32, name=f"of{i}",
                                tag=f"of{i}") for i in range(CT)]
                for hf in range(2):
                    layer_norm(z2, lng["g2"], lng["be2"], outf, None,
                               (ps_ln2, lnp2), out_dram=out,
                               col=slice(hf * (TSL // 2),
                                         (hf + 1) * (TSL // 2)))

        for rep in range(reps):
            one_rep(rep)


def _build(causal: bool, masked: bool, sim: bool = False, reps: int = 1):
    kopts = tuple(sorted((k, v) for k, v in os.environ.items()
                         if k.startswith(("KOPT_", "KERNEL_NWARM"))))
    key = (causal, masked, sim, reps, kopts)
    if key in _CACHE:
        return _CACHE[key]
    nc = bacc.Bacc("TRN2", target_bir_lowering=False, debug=False,
                   num_devices=NCORES)
    _emit(nc, causal, masked, sim=sim, reps=reps)
    nc.compile()
    _CACHE[key] = nc
    return nc


def kernel(x, attention_mask, Wq, bq, Wk, bk, Wv, bv, W1, b1, W2, b2,
           g1, be1, g2, be2):
    global LAST_EXEC_NS
    f32 = np.float32
    x = np.asarray(x, f32).reshape(TOK, C)
    xT = np.ascontiguousarray(x.T)                      # [C, TOK]
    mask = np.asarray(attention_mask).reshape(T, T)

    causal = bool(np.array_equal(
        mask != 0, np.tril(np.ones((T, T), dtype=bool))))
    masked = (not causal) and not bool((mask != 0).all())

    Wq = np.asarray(Wq, f32); Wk = np.asarray(Wk, f32)
    Wv = np.asarray(Wv, f32); W1 = np.asarray(W1, f32)
    W2 = np.asarray(W2, f32)
    bq = np.asarray(bq, f32); bk = np.asarray(bk, f32)
    bv = np.asarray(bv, f32); b1 = np.asarray(b1, f32)
    b2 = np.asarray(b2, f32)
    scale = 1.0 / math.sqrt(DK)

    shared = dict(
        xT=xT.reshape(CT, 128, TOK).astype(BF),
        w1=W1.astype(BF),
        b1=np.ascontiguousarray(b1.reshape(DFF // 128, 128).T),
        w2=W2.astype(BF),
        b2x2=np.ascontiguousarray((2.0 * b2).reshape(CT, 128).T),
        g1v=np.ascontiguousarray(np.asarray(g1, f32).reshape(CT, 128).T),
        be1v=np.ascontiguousarray(np.asarray(be1, f32).reshape(CT, 128).T),
        g2v=np.ascontiguousarray(np.asarray(g2, f32).reshape(CT, 128).T),
        be2v=np.ascontiguousarray(np.asarray(be2, f32).reshape(CT, 128).T),
    )
    if masked:
        add = np.where(mask != 0, 0.0, -30000.0).astype(f32)
        shared["amask"] = np.ascontiguousarray(add.T).reshape(
            T // 128, 128, T).astype(BF)

    in_maps = []
    for c in range(NCORES):
        hs = slice(c * 128, (c + 1) * 128)
        m = dict(shared)
        m["xres"] = np.ascontiguousarray(
            xT[:, c * TSL:(c + 1) * TSL]).reshape(CT, 128, TSL)
        m["wq"] = np.ascontiguousarray(
            Wq[:, hs] * scale).reshape(CT, 128, 128).astype(BF)
        m["wk"] = np.ascontiguousarray(Wk[:, hs]).reshape(
            CT, 128, 128).astype(BF)
        m["wv"] = np.ascontiguousarray(Wv[:, hs]).reshape(
            CT, 128, 128).astype(BF)
        m["bqkv"] = np.ascontiguousarray(
            np.stack([bq[hs] * scale, bk[hs], bv[hs]], axis=1)).astype(f32)
        in_maps.append(m)

    reps = int(os.environ.get("KERNEL_REPS", "1"))
    nc = _build(causal, masked, reps=reps)
    res = run_bass_kernel_spmd(nc, in_maps, list(range(NCORES)))
    LAST_EXEC_NS = res.exec_time_ns

    outT = np.concatenate(
        [res.results[c]["out"].reshape(C, TSL) for c in range(NCORES)],
        axis=1)                                          # [C, TOK]
    return np.ascontiguousarray(outT.T).reshape(B, T, C).astype(f32)

